# revision 9
# baseline (speedup 1.0000x reference)
"""DPLSTMCell Trainium2 kernel — per-gate mixed precision (fp8 + fp16).

Data-parallel LSTM cell over 8 NeuronCores: batch dim of input/h_prev/c_prev
is sharded, the (small) weights are replicated.

Precision scheme (error budget rel<2e-2; measured rel_h~=1.62e-2):
  gates i,f,o use fp8e4m3 DoubleRow matmuls (contract 256 k-rows per
  instruction instead of 128), the tanh-gate g stays fp16.  All-fp8
  measures 2.57e-2 (fails), so this mix is the PE-time floor: the PE
  streams 1 psum column per 2.4GHz cycle, giving
    rows = 8 b-tiles * (8*768 fp8 + 16*256 fp16) * 4 quarters
         = 327680 rows ~= 136.5us/core.
  Operands are pre-scaled by powers of two (x*8, W*512, exact in fp16) so
  fp8 stays clear of subnormals while psum+bias (max ~2e4) fits fp16:
  the gates tile is fp16, halving DVE/ACT width in the epilogue.  ACT
  descales by 2^-12 via its scale immediate.

Schedule (v5) — driven by NTFF traces (baseline 177.4us):
  - Quarter 0 was DMA-bound (~10MB of input in its ~35us window at
    ~330GB/s).  xh8 is not shipped at all: only xh16 loads; the DVE
    casts fp16->fp8e4 on-chip (~1.2us per b-tile, bit-exact RTN), saving
    2.1MB of head-critical DMA.  xh16 and the DoubleRow fp8 layout share
    the same linear SBUF layout so the cast is elementwise.
  - bias is loaded as one [1, 4H] f32 row (16KB) and partition-broadcast
    by the idle Pool engine, replacing a 2MB replicated load.
  - b0/b1 loads and casts are split in k-halves so the first real
    matmuls start as soon as ~700KB has landed.
  - Quarter 0 runs pairwise (GRP=2): with 4 PSUM buffers two pairs are
    in flight, so pair N+1's fp8 overlaps pair N's fp16/bias_add drain.
  - Engine instruction streams are in-order: a stalled op blocks
    everything behind it.  DMA dispatch order matches consumption order
    on both queues; the next quarter's W prefetch is emitted before this
    quarter's output DMAs on the sync queue; fp8 casts are emitted two
    pairs ahead on the DVE stream.
  - ch_out is tile-major in DRAM: each epilogue writes one contiguous
    128KB block (vs 256 strided 512B lines); host reassembles.
"""

import numpy as np
import ml_dtypes

import concourse.bacc as bacc
import concourse.mybir as mybir
import concourse.tile as tile
from concourse.bass_utils import run_bass_kernel_spmd

AF = mybir.ActivationFunctionType
DR = mybir.MatmulPerfMode.DoubleRow
F8 = mybir.dt.float8e4
F16 = mybir.dt.float16
F32 = mybir.dt.float32

N_CORES = 8
B_TOTAL = 8192
IN_DIM = 1024
H_DIM = 1024
P = 128

SX = 8.0         # x pre-scale (power of two)
SW = 512.0       # W pre-scale (power of two)
INV = 1.0 / (SX * SW)   # 2^-12, exact


def build_lstm_nc(b_loc=B_TOTAL // N_CORES, in_dim=IN_DIM, h_dim=H_DIM):
    ktot = in_dim + h_dim
    KT16 = ktot // P            # fp16 k-tiles (g gate)
    KT8 = ktot // (2 * P)       # fp8 DoubleRow k-super-tiles (i,f,o gates)
    G = 4 * h_dim               # total gate width
    NQ = 4                      # quarters, each [i|f|o|g] x DS
    QW = G // NQ                # quarter width (1024)
    DS = h_dim // NQ            # output-dim slice per quarter (256)
    W8C = 3 * DS                # fp8 cols per quarter (768: i,f,o)
    BT = b_loc // P             # batch tiles per core (8)

    nc = bacc.Bacc("TRN2", target_bir_lowering=False)
    # PE-ready host layouts; leading dim = SBUF partition (contraction k%128)
    xh16 = nc.dram_tensor("xh16", [P, BT, KT16, P], F16, kind="ExternalInput")
    w16 = nc.dram_tensor("w16", [NQ, P, KT16, DS], F16, kind="ExternalInput")
    w8 = nc.dram_tensor("w8", [NQ, P, KT8, 2, W8C], F8, kind="ExternalInput")
    bias = nc.dram_tensor("bias", [1, G], F32, kind="ExternalInput")
    c_prev = nc.dram_tensor("c_prev", [P, BT, NQ, DS], F16,
                            kind="ExternalInput")
    # tile-major output: epilogue (q,b) writes ch_out[b,q] as one
    # contiguous 128KB block; host reassembles to [b_loc, 2, h].
    ch_out = nc.dram_tensor("ch_out", [BT, NQ, P, 2, DS], F16,
                            kind="ExternalOutput")

    with tile.TileContext(nc) as tc:
        with (
            tc.tile_pool(name="main", bufs=1) as main_pool,
            tc.tile_pool(name="w8p", bufs=2) as w8_pool,
            tc.tile_pool(name="w16p", bufs=2) as w16_pool,
            tc.tile_pool(name="psum", bufs=4, space="PSUM") as psum_pool,
        ):
            xh16_sb = main_pool.tile([P, BT, KT16, P], F16)
            # fp8 copy of xh16, same linear layout; [:, b, 2t:2t+2, :] is
            # the DoubleRow stationary slice.
            xh8_sb = main_pool.tile([P, BT, KT16, P], F8)
            cp_sb = main_pool.tile([P, BT, NQ, DS], F16)
            bias_row = main_pool.tile([1, G], F32)
            bias_sb = main_pool.tile([P, G], F32)
            w8_tiles = {}
            w16_tiles = {}

            def alloc_w_quarter(q):
                w8_tiles[q] = w8_pool.tile([P, KT8, 2, W8C], F8, name="w8q")
                w16_tiles[q] = w16_pool.tile([P, KT16, DS], F16, name="w16q")

            def load_w_quarter(q):
                nc.sync.dma_start(w8_tiles[q][:], w8[q, :, :, :, :])
                nc.sync.dma_start(w16_tiles[q][:], w16[q, :, :, :])

            # ---- DMA dispatch plan ----------------------------------------
            # sync (hw DGE): bias row + W stream, in consumption order.
            # gpsimd (sw DGE): xh16 + c_prev, staggered to their use times.
            alloc_w_quarter(0)
            nc.sync.dma_start(bias_row[:], bias[:, :])
            nc.gpsimd.dma_start(xh16_sb[:, 0, 0:8], xh16[:, 0, 0:8])
            nc.sync.dma_start(w8_tiles[0][:, 0:2], w8[0, :, 0:2, :, :])
            nc.gpsimd.dma_start(xh16_sb[:, 1, 0:8], xh16[:, 1, 0:8])
            nc.sync.dma_start(w8_tiles[0][:, 2:5], w8[0, :, 2:5, :, :])
            nc.gpsimd.dma_start(xh16_sb[:, 0, 8:KT16], xh16[:, 0, 8:KT16])
            nc.gpsimd.dma_start(xh16_sb[:, 1, 8:KT16], xh16[:, 1, 8:KT16])
            nc.sync.dma_start(w8_tiles[0][:, 5:KT8], w8[0, :, 5:KT8, :, :])
            nc.gpsimd.dma_start(xh16_sb[:, 2], xh16[:, 2])
            nc.sync.dma_start(w16_tiles[0][:, 0:8], w16[0, :, 0:8, :])
            nc.gpsimd.dma_start(xh16_sb[:, 3], xh16[:, 3])
            nc.sync.dma_start(w16_tiles[0][:, 8:KT16], w16[0, :, 8:KT16, :])
            nc.gpsimd.dma_start(xh16_sb[:, 4:6], xh16[:, 4:6])
            nc.gpsimd.dma_start(cp_sb[:, 0:2], c_prev[:, 0:2])
            nc.gpsimd.dma_start(xh16_sb[:, 6:BT], xh16[:, 6:BT])
            nc.gpsimd.dma_start(cp_sb[:, 2:5], c_prev[:, 2:5])
            nc.gpsimd.dma_start(cp_sb[:, 5:BT], c_prev[:, 5:BT])
            # quarter-1 W right behind quarter 0's (plenty of hw-queue slack
            # once xh8 no longer ships).
            alloc_w_quarter(1)
            load_w_quarter(1)

            # bias broadcast on the otherwise-idle Pool engine.
            for q in range(NQ):
                nc.gpsimd.partition_broadcast(
                    bias_sb[:, q * QW:(q + 1) * QW],
                    bias_row[0:1, q * QW:(q + 1) * QW])

            # fp16->fp8 casts on the DVE (~1.2us per b-tile, bit-exact RTN),
            # emitted ahead of need; b0/b1 by k-halves for the fastest start.
            def cast_b(b, lo=0, hi=KT16):
                nc.vector.tensor_copy(xh8_sb[:, b, lo:hi],
                                      xh16_sb[:, b, lo:hi])

            cast_b(0, 0, 8)
            cast_b(1, 0, 8)
            cast_b(0, 8, KT16)
            cast_b(1, 8, KT16)
            cast_b(2)
            cast_b(3)

            # PE warmup: dummy matmuls on zeroed SBUF while the first W/xh
            # tiles stream in, so the PE p-state ramps before real work.
            scratch = main_pool.tile([P, 512], F16, name="scratch")
            nc.vector.memset(scratch[:], 0.0)
            zb = main_pool.tile([P, 1], F32)
            nc.vector.memset(zb[:], 0.0)
            ps_w = psum_pool.tile([P, QW], F32, name="ps")
            for i in range(8):
                nc.tensor.matmul(
                    ps_w[:, (i % 2) * 512:(i % 2) * 512 + 512],
                    scratch[:, 0:P], scratch[:],
                    start=True, stop=True)

            def mm_fp8(ps, q, t, b):
                # i|f chunk (cols 0:512, psum bank A) and o chunk (512:768)
                w8_q = w8_tiles[q]
                xsl = xh8_sb[:, b, 2 * t:2 * t + 2, :]
                nc.tensor.matmul(ps[:, 0:512], xsl, w8_q[:, t, :, 0:512],
                                 perf_mode=DR,
                                 start=(t == 0), stop=(t == KT8 - 1))
                nc.tensor.matmul(ps[:, 512:W8C], xsl, w8_q[:, t, :, 512:W8C],
                                 perf_mode=DR,
                                 start=(t == 0), stop=(t == KT8 - 1))

            def mm_fp16(ps, q, k, b):
                # g chunk (cols 768:1024, psum bank B)
                nc.tensor.matmul(ps[:, W8C:QW],
                                 xh16_sb[:, b, k, :],
                                 w16_tiles[q][:, k, :],
                                 start=(k == 0), stop=(k == KT16 - 1))

            def epilogue(ps, q, b):
                # quarter layout: [ i | f | o | g ], each DS wide; the fp16
                # gates tile (values fit: max ~2e4 < 65504) halves DVE/ACT
                # width; ACT descales by 2^-12 via its scale immediate.
                # gates = psum + bias on the DVE; the ONLY psum reader, so
                # the PSUM slot frees right after it.
                gates = work_tile([P, QW], F16, "gates")
                nc.vector.tensor_add(
                    gates[:], ps[:], bias_sb[:, q * QW:(q + 1) * QW])
                act = work_tile([P, QW], F16, "act")
                nc.scalar.activation(act[:, 3 * DS:4 * DS],
                                     gates[:, 3 * DS:4 * DS], AF.Tanh,
                                     bias=zb[:], scale=INV)
                nc.scalar.activation(act[:, 0:3 * DS], gates[:, 0:3 * DS],
                                     AF.Sigmoid, bias=zb[:], scale=INV)

                ig = work_tile([P, DS], F16, "ig")
                nc.vector.tensor_mul(ig[:], act[:, 0:DS],
                                     act[:, 3 * DS:4 * DS])
                chnew = work_tile([P, 2, DS], F16, "chnew")
                cnew = chnew[:, 0, :]
                nc.vector.tensor_mul(cnew, act[:, DS:2 * DS],
                                     cp_sb[:, b, q, :])
                nc.vector.tensor_add(cnew, cnew, ig[:])
                tct = work_tile([P, DS], F16, "tct")
                nc.scalar.activation(tct[:], cnew, AF.Tanh, bias=zb[:])
                nc.vector.tensor_mul(chnew[:, 1, :], act[:, 2 * DS:3 * DS],
                                     tct[:])
                nc.sync.dma_start(ch_out[b, q], chnew[:, :, :])

            def work_tile(shape, dt, name):
                return main_pool.tile(shape, dt, name=name, bufs=8)

            # ---- quarter 0: pairwise (2 psum tiles/pair, 4 bufs = 2 pairs
            # in flight): pair N+1's fp8 overlaps pair N's fp16 drain ----
            for g0 in range(0, BT, 2):
                pss = [psum_pool.tile([P, QW], F32, name="ps")
                       for _ in range(2)]
                for t in range(KT8):
                    for bi, ps in enumerate(pss):
                        mm_fp8(ps, 0, t, g0 + bi)
                for bi, ps in enumerate(pss):
                    for k in range(KT16):
                        mm_fp16(ps, 0, k, g0 + bi)
                # stay two pairs ahead on the fp8 casts (DVE in-order:
                # these run before the pair's epilogue adds)
                for b in range(g0 + 4, min(g0 + 6, BT)):
                    cast_b(b)
                for bi, ps in enumerate(pss):
                    epilogue(ps, 0, g0 + bi)

            # ---- quarters 1..3: prefetched, dense per-b chains; next
            # quarter's W dispatch is emitted BEFORE this quarter's output
            # DMAs so the sync engine's in-order stream never blocks the
            # prefetch behind epilogue semaphores ----
            for q in range(1, NQ):
                if q + 1 < NQ:
                    alloc_w_quarter(q + 1)
                    load_w_quarter(q + 1)
                for b in range(BT):
                    ps = psum_pool.tile([P, QW], F32, name="ps")
                    for t in range(KT8):
                        mm_fp8(ps, q, t, b)
                    for k in range(KT16):
                        mm_fp16(ps, q, k, b)
                    epilogue(ps, q, b)

    nc.compile()
    return nc


def prep_inputs(input, h_prev, c_prev, W_ih, b_ih, W_hh, b_hh,
                n_cores=N_CORES):
    """Host-side shard + quantize + layout prep. Per-core input maps."""
    input = np.asarray(input, np.float32)
    h_prev = np.asarray(h_prev, np.float32)
    c_prev = np.asarray(c_prev, np.float32)
    W_ih = np.asarray(W_ih, np.float32)
    W_hh = np.asarray(W_hh, np.float32)
    b_ih = np.asarray(b_ih, np.float32)
    b_hh = np.asarray(b_hh, np.float32)

    b_total, in_dim = input.shape
    h_dim = h_prev.shape[1]
    ktot = in_dim + h_dim
    b_loc = b_total // n_cores
    G = 4 * h_dim
    NQ = 4
    DS = h_dim // NQ
    W8C = 3 * DS
    BT = b_loc // 128
    KT16 = ktot // 128
    KT8 = ktot // 256

    def q8(x):
        return np.clip(x, -240, 240).astype(ml_dtypes.float8_e4m3)

    # column reorder: per quarter q the layout is [i | f | o | g] for output
    # dims [q*DS, (q+1)*DS)
    arr = np.arange(G).reshape(4, NQ, DS)       # [gate, q, r]
    idx = arr[[0, 1, 3, 2]].transpose(1, 0, 2).reshape(-1)

    W_cat = np.concatenate([W_ih, W_hh], axis=1)[idx, :]    # [G, ktot] scaled
    Ws = W_cat * SW
    # fp8 blocks (i,f,o = first 768 cols of each quarter) in DoubleRow layout
    w8_host = np.empty((NQ, 128, KT8, 2, W8C), ml_dtypes.float8_e4m3)
    w16_host = np.empty((NQ, 128, KT16, DS), np.float16)
    for q in range(NQ):
        blk8 = q8(Ws[q * 1024:q * 1024 + W8C, :]).T         # [ktot, 768]
        w8_host[q] = blk8.reshape(KT8, 2, 128, W8C).transpose(2, 0, 1, 3)
        blk16 = Ws[q * 1024 + W8C:(q + 1) * 1024, :].T.astype(np.float16)
        w16_host[q] = blk16.reshape(KT16, 128, DS).transpose(1, 0, 2)

    bias_host = ((b_ih + b_hh)[idx] * (SX * SW)).astype(np.float32)
    bias_host = np.ascontiguousarray(bias_host.reshape(1, G))

    xh = np.concatenate([input, h_prev], axis=1) * SX       # [B, ktot] scaled
    x16 = xh.astype(np.float16)
    c16 = c_prev.astype(np.float16)

    in_maps = []
    for c in range(n_cores):
        rows = slice(c * b_loc, (c + 1) * b_loc)
        xc16 = x16[rows].T                                  # [ktot, b_loc]
        # [p, b, k, m] = x[k*128 + p, b*128 + m]
        xh16_h = xc16.reshape(KT16, 128, BT, 128).transpose(1, 2, 0, 3)
        in_maps.append({
            "xh16": np.ascontiguousarray(xh16_h),
            "w8": w8_host,
            "w16": w16_host,
            "bias": bias_host,
            "c_prev": np.ascontiguousarray(
                c16[rows].reshape(BT, 128, NQ, DS).transpose(1, 0, 2, 3)),
        })
    return in_maps


def unshard_output(results, b_loc=B_TOTAL // N_CORES, h_dim=H_DIM):
    """ch_out [BT, NQ, 128, 2, DS] tile-major -> full (h_t, c_t) fp32."""
    NQ = 4
    DS = h_dim // NQ
    BT = b_loc // 128
    chs = []
    for r in results:
        t = r["ch_out"].reshape(BT, NQ, 128, 2, DS)
        # -> [BT, 128, 2, NQ, DS] -> [b_loc, 2, h]
        chs.append(t.transpose(0, 2, 3, 1, 4).reshape(b_loc, 2, h_dim))
    ch = np.concatenate(chs, axis=0)
    c_t = ch[:, 0, :].astype(np.float32)
    h_t = ch[:, 1, :].astype(np.float32)
    return h_t, c_t


def run_lstm(inputs, trace=False, **spmd_kwargs):
    """Builds + runs the kernel on all 8 cores. Returns (h_t, c_t), results."""
    in_maps = prep_inputs(**inputs)
    nc = build_lstm_nc()
    res = run_bass_kernel_spmd(nc, in_maps, core_ids=list(range(N_CORES)),
                               trace=trace, **spmd_kwargs)
    h_t, c_t = unshard_output(res.results)
    return (h_t, c_t), res


def kernel(input, h_prev, c_prev, W_ih, b_ih, W_hh, b_hh):
    (h_t, c_t), _ = run_lstm(dict(
        input=input, h_prev=h_prev, c_prev=c_prev,
        W_ih=W_ih, b_ih=b_ih, W_hh=W_hh, b_hh=b_hh))
    return (h_t, c_t)


# revision 11
# speedup vs baseline: 1.1511x; 1.1511x over previous
"""DPLSTMCell Trainium2 kernel — per-gate mixed precision (fp8 + fp16).

Data-parallel LSTM cell over 8 NeuronCores: batch dim of input/h_prev/c_prev
is sharded, the (small) weights are replicated.

Precision scheme (error budget rel<2e-2; measured rel_h~=1.62e-2):
  gates i,f,o use fp8e4m3 DoubleRow matmuls (contract 256 k-rows per
  instruction instead of 128), the tanh-gate g stays fp16.  All-fp8
  measures 2.57e-2 (fails), so this mix is the PE-time floor: the PE
  streams 1 psum column per 2.4GHz cycle, giving
    rows = 8 b-tiles * (8*768 fp8 + 16*256 fp16) * 4 quarters
         = 327680 rows ~= 136.5us/core.
  Operands are pre-scaled by powers of two (x*8, W*512, exact in fp16) so
  fp8 stays clear of subnormals while psum+bias (max ~2e4) fits fp16:
  the gates tile is fp16, halving DVE/ACT width in the epilogue.  ACT
  descales by 2^-12 via its scale immediate.

Schedule (v5) — driven by NTFF traces (baseline 177.4us):
  - Quarter 0 was DMA-bound (~10MB of input in its ~35us window at
    ~330GB/s).  xh8 is not shipped at all: only xh16 loads; the DVE
    casts fp16->fp8e4 on-chip (~1.2us per b-tile, bit-exact RTN), saving
    2.1MB of head-critical DMA.  xh16 and the DoubleRow fp8 layout share
    the same linear SBUF layout so the cast is elementwise.
  - bias is loaded as one [1, 4H] f32 row (16KB) and partition-broadcast
    by the idle Pool engine, replacing a 2MB replicated load.
  - b0/b1 loads and casts are split in k-halves so the first real
    matmuls start as soon as ~700KB has landed.
  - Quarter 0 runs pairwise (GRP=2): with 4 PSUM buffers two pairs are
    in flight, so pair N+1's fp8 overlaps pair N's fp16/bias_add drain.
  - Engine instruction streams are in-order: a stalled op blocks
    everything behind it.  DMA dispatch order matches consumption order
    on both queues; the next quarter's W prefetch is emitted before this
    quarter's output DMAs on the sync queue; fp8 casts are emitted two
    pairs ahead on the DVE stream.
  - ch_out is tile-major in DRAM: each epilogue writes one contiguous
    128KB block (vs 256 strided 512B lines); host reassembles.
"""

import numpy as np
import ml_dtypes

import concourse.bacc as bacc
import concourse.mybir as mybir
import concourse.tile as tile
from concourse.bass_utils import run_bass_kernel_spmd

AF = mybir.ActivationFunctionType
DR = mybir.MatmulPerfMode.DoubleRow
F8 = mybir.dt.float8e4
F16 = mybir.dt.float16
F32 = mybir.dt.float32

N_CORES = 8
B_TOTAL = 8192
IN_DIM = 1024
H_DIM = 1024
P = 128

SX = 8.0         # x pre-scale (power of two)
SW = 512.0       # W pre-scale (power of two)
INV = 1.0 / (SX * SW)   # 2^-12, exact


def build_lstm_nc(b_loc=B_TOTAL // N_CORES, in_dim=IN_DIM, h_dim=H_DIM):
    ktot = in_dim + h_dim
    KT16 = ktot // P            # fp16 k-tiles (g gate)
    KT8 = ktot // (2 * P)       # fp8 DoubleRow k-super-tiles (i,f,o gates)
    G = 4 * h_dim               # total gate width
    NQ = 4                      # quarters, each [i|f|o|g] x DS
    QW = G // NQ                # quarter width (1024)
    DS = h_dim // NQ            # output-dim slice per quarter (256)
    W8C = 3 * DS                # fp8 cols per quarter (768: i,f,o)
    BT = b_loc // P             # batch tiles per core (8)

    nc = bacc.Bacc("TRN2", target_bir_lowering=False)
    # PE-ready host layouts; leading dim = SBUF partition (contraction k%128)
    xh16 = nc.dram_tensor("xh16", [P, BT, KT16, P], F16, kind="ExternalInput")
    w16 = nc.dram_tensor("w16", [NQ, P, KT16, DS], F16, kind="ExternalInput")
    w8 = nc.dram_tensor("w8", [NQ, P, KT8, 2, W8C], F8, kind="ExternalInput")
    bias = nc.dram_tensor("bias", [1, G], F32, kind="ExternalInput")
    c_prev = nc.dram_tensor("c_prev", [P, BT, NQ, DS], F16,
                            kind="ExternalInput")
    # tile-major output: epilogue (q,b) writes ch_out[b,q] as one
    # contiguous 128KB block; host reassembles to [b_loc, 2, h].
    ch_out = nc.dram_tensor("ch_out", [BT, NQ, P, 2, DS], F16,
                            kind="ExternalOutput")

    with tile.TileContext(nc) as tc:
        with (
            tc.tile_pool(name="const", bufs=1) as const_pool,
            tc.tile_pool(name="xh", bufs=1) as xh_pool,
            tc.tile_pool(name="w8p", bufs=2) as w8_pool,
            tc.tile_pool(name="w16p", bufs=2) as w16_pool,
            tc.tile_pool(name="work", bufs=3) as work,
            tc.tile_pool(name="psum", bufs=4, space="PSUM") as psum_pool,
        ):
            xh16_sb = xh_pool.tile([P, BT, KT16, P], F16)
            # fp8 copy of xh16, same linear layout; [:, b, 2t:2t+2, :] is
            # the DoubleRow stationary slice.
            xh8_sb = xh_pool.tile([P, BT, KT16, P], F8)
            cp_sb = xh_pool.tile([P, BT, NQ, DS], F16)
            bias_row = const_pool.tile([1, G], F32)
            bias_sb = const_pool.tile([P, G], F32)
            w8_tiles = {}
            w16_tiles = {}

            def alloc_w_quarter(q):
                w8_tiles[q] = w8_pool.tile([P, KT8, 2, W8C], F8, name="w8q")
                w16_tiles[q] = w16_pool.tile([P, KT16, DS], F16, name="w16q")

            def load_w_quarter(q):
                nc.sync.dma_start(w8_tiles[q][:], w8[q, :, :, :, :])
                nc.sync.dma_start(w16_tiles[q][:], w16[q, :, :, :])

            # ---- DMA dispatch plan ----------------------------------------
            # sync (hw DGE): bias row + W stream, in consumption order.
            # gpsimd (sw DGE): xh16 + c_prev, staggered to their use times.
            alloc_w_quarter(0)
            nc.sync.dma_start(bias_row[:], bias[:, :])
            nc.gpsimd.dma_start(xh16_sb[:, 0, 0:8], xh16[:, 0, 0:8])
            nc.sync.dma_start(w8_tiles[0][:, 0:2], w8[0, :, 0:2, :, :])
            nc.gpsimd.dma_start(xh16_sb[:, 1, 0:8], xh16[:, 1, 0:8])
            nc.sync.dma_start(w8_tiles[0][:, 2:5], w8[0, :, 2:5, :, :])
            nc.gpsimd.dma_start(xh16_sb[:, 0, 8:KT16], xh16[:, 0, 8:KT16])
            nc.gpsimd.dma_start(xh16_sb[:, 1, 8:KT16], xh16[:, 1, 8:KT16])
            nc.sync.dma_start(w8_tiles[0][:, 5:KT8], w8[0, :, 5:KT8, :, :])
            nc.gpsimd.dma_start(xh16_sb[:, 2], xh16[:, 2])
            nc.sync.dma_start(w16_tiles[0][:, 0:8], w16[0, :, 0:8, :])
            nc.gpsimd.dma_start(xh16_sb[:, 3], xh16[:, 3])
            nc.sync.dma_start(w16_tiles[0][:, 8:KT16], w16[0, :, 8:KT16, :])
            nc.gpsimd.dma_start(xh16_sb[:, 4:6], xh16[:, 4:6])
            nc.gpsimd.dma_start(cp_sb[:, 0:2], c_prev[:, 0:2])
            nc.gpsimd.dma_start(xh16_sb[:, 6:BT], xh16[:, 6:BT])
            nc.gpsimd.dma_start(cp_sb[:, 2:5], c_prev[:, 2:5])
            nc.gpsimd.dma_start(cp_sb[:, 5:BT], c_prev[:, 5:BT])
            # quarter-1 W right behind quarter 0's (plenty of hw-queue slack
            # once xh8 no longer ships).
            alloc_w_quarter(1)
            load_w_quarter(1)

            # bias broadcast on the otherwise-idle Pool engine.
            for q in range(NQ):
                nc.gpsimd.partition_broadcast(
                    bias_sb[:, q * QW:(q + 1) * QW],
                    bias_row[0:1, q * QW:(q + 1) * QW])

            # fp16->fp8 casts on the DVE (~1.2us per b-tile, bit-exact RTN),
            # emitted ahead of need; b0/b1 by k-halves for the fastest start.
            def cast_b(b, lo=0, hi=KT16):
                nc.vector.tensor_copy(xh8_sb[:, b, lo:hi],
                                      xh16_sb[:, b, lo:hi])

            cast_b(0, 0, 8)
            cast_b(1, 0, 8)
            cast_b(0, 8, KT16)
            cast_b(1, 8, KT16)
            cast_b(2)
            cast_b(3)

            # PE warmup: dummy matmuls on zeroed SBUF while the first W/xh
            # tiles stream in, so the PE p-state ramps before real work.
            scratch = work.tile([P, 512], F16, name="scratch", bufs=1)
            nc.vector.memset(scratch[:], 0.0)
            zb = const_pool.tile([P, 1], F32)
            nc.vector.memset(zb[:], 0.0)
            ps_w = psum_pool.tile([P, QW], F32, name="ps")
            for i in range(8):
                nc.tensor.matmul(
                    ps_w[:, (i % 2) * 512:(i % 2) * 512 + 512],
                    scratch[:, 0:P], scratch[:],
                    start=True, stop=True)

            def mm_fp8(ps, q, t, b):
                # i|f chunk (cols 0:512, psum bank A) and o chunk (512:768)
                w8_q = w8_tiles[q]
                xsl = xh8_sb[:, b, 2 * t:2 * t + 2, :]
                nc.tensor.matmul(ps[:, 0:512], xsl, w8_q[:, t, :, 0:512],
                                 perf_mode=DR,
                                 start=(t == 0), stop=(t == KT8 - 1))
                nc.tensor.matmul(ps[:, 512:W8C], xsl, w8_q[:, t, :, 512:W8C],
                                 perf_mode=DR,
                                 start=(t == 0), stop=(t == KT8 - 1))

            def mm_fp16(ps, q, k, b):
                # g chunk (cols 768:1024, psum bank B)
                nc.tensor.matmul(ps[:, W8C:QW],
                                 xh16_sb[:, b, k, :],
                                 w16_tiles[q][:, k, :],
                                 start=(k == 0), stop=(k == KT16 - 1))

            def epilogue(ps, q, b):
                # quarter layout: [ i | f | o | g ], each DS wide; the fp16
                # gates tile (values fit: max ~2e4 < 65504) halves DVE/ACT
                # width; ACT descales by 2^-12 via its scale immediate.
                # gates = psum + bias on the DVE; the ONLY psum reader, so
                # the PSUM slot frees right after it.
                gates = work_tile([P, QW], F16, "gates")
                nc.vector.tensor_add(
                    gates[:], ps[:], bias_sb[:, q * QW:(q + 1) * QW])
                act = work_tile([P, QW], F16, "act")
                nc.scalar.activation(act[:, 3 * DS:4 * DS],
                                     gates[:, 3 * DS:4 * DS], AF.Tanh,
                                     bias=zb[:], scale=INV)
                nc.scalar.activation(act[:, 0:3 * DS], gates[:, 0:3 * DS],
                                     AF.Sigmoid, bias=zb[:], scale=INV)

                ig = work_tile([P, DS], F16, "ig")
                nc.vector.tensor_mul(ig[:], act[:, 0:DS],
                                     act[:, 3 * DS:4 * DS])
                chnew = work_tile([P, 2, DS], F16, "chnew")
                cnew = chnew[:, 0, :]
                nc.vector.tensor_mul(cnew, act[:, DS:2 * DS],
                                     cp_sb[:, b, q, :])
                nc.vector.tensor_add(cnew, cnew, ig[:])
                tct = work_tile([P, DS], F16, "tct")
                nc.scalar.activation(tct[:], cnew, AF.Tanh, bias=zb[:])
                nc.vector.tensor_mul(chnew[:, 1, :], act[:, 2 * DS:3 * DS],
                                     tct[:])
                nc.sync.dma_start(ch_out[b, q], chnew[:, :, :])

            def work_tile(shape, dt, name):
                return work.tile(shape, dt, name=name, bufs=8)

            # ---- quarter 0: pairwise (2 psum tiles/pair, 4 bufs = 2 pairs
            # in flight): pair N+1's fp8 overlaps pair N's fp16 drain ----
            for g0 in range(0, BT, 2):
                pss = [psum_pool.tile([P, QW], F32, name="ps")
                       for _ in range(2)]
                for t in range(KT8):
                    for bi, ps in enumerate(pss):
                        mm_fp8(ps, 0, t, g0 + bi)
                for bi, ps in enumerate(pss):
                    for k in range(KT16):
                        mm_fp16(ps, 0, k, g0 + bi)
                # stay two pairs ahead on the fp8 casts (DVE in-order:
                # these run before the pair's epilogue adds)
                for b in range(g0 + 4, min(g0 + 6, BT)):
                    cast_b(b)
                for bi, ps in enumerate(pss):
                    epilogue(ps, 0, g0 + bi)

            # ---- quarters 1..3: prefetched, dense per-b chains; next
            # quarter's W dispatch is emitted BEFORE this quarter's output
            # DMAs so the sync engine's in-order stream never blocks the
            # prefetch behind epilogue semaphores ----
            for q in range(1, NQ):
                if q + 1 < NQ:
                    alloc_w_quarter(q + 1)
                    load_w_quarter(q + 1)
                for b in range(BT):
                    ps = psum_pool.tile([P, QW], F32, name="ps")
                    for t in range(KT8):
                        mm_fp8(ps, q, t, b)
                    for k in range(KT16):
                        mm_fp16(ps, q, k, b)
                    epilogue(ps, q, b)

    nc.compile()
    return nc


def prep_inputs(input, h_prev, c_prev, W_ih, b_ih, W_hh, b_hh,
                n_cores=N_CORES):
    """Host-side shard + quantize + layout prep. Per-core input maps."""
    input = np.asarray(input, np.float32)
    h_prev = np.asarray(h_prev, np.float32)
    c_prev = np.asarray(c_prev, np.float32)
    W_ih = np.asarray(W_ih, np.float32)
    W_hh = np.asarray(W_hh, np.float32)
    b_ih = np.asarray(b_ih, np.float32)
    b_hh = np.asarray(b_hh, np.float32)

    b_total, in_dim = input.shape
    h_dim = h_prev.shape[1]
    ktot = in_dim + h_dim
    b_loc = b_total // n_cores
    G = 4 * h_dim
    NQ = 4
    DS = h_dim // NQ
    W8C = 3 * DS
    BT = b_loc // 128
    KT16 = ktot // 128
    KT8 = ktot // 256

    def q8(x):
        return np.clip(x, -240, 240).astype(ml_dtypes.float8_e4m3)

    # column reorder: per quarter q the layout is [i | f | o | g] for output
    # dims [q*DS, (q+1)*DS)
    arr = np.arange(G).reshape(4, NQ, DS)       # [gate, q, r]
    idx = arr[[0, 1, 3, 2]].transpose(1, 0, 2).reshape(-1)

    W_cat = np.concatenate([W_ih, W_hh], axis=1)[idx, :]    # [G, ktot] scaled
    Ws = W_cat * SW
    # fp8 blocks (i,f,o = first 768 cols of each quarter) in DoubleRow layout
    w8_host = np.empty((NQ, 128, KT8, 2, W8C), ml_dtypes.float8_e4m3)
    w16_host = np.empty((NQ, 128, KT16, DS), np.float16)
    for q in range(NQ):
        blk8 = q8(Ws[q * 1024:q * 1024 + W8C, :]).T         # [ktot, 768]
        w8_host[q] = blk8.reshape(KT8, 2, 128, W8C).transpose(2, 0, 1, 3)
        blk16 = Ws[q * 1024 + W8C:(q + 1) * 1024, :].T.astype(np.float16)
        w16_host[q] = blk16.reshape(KT16, 128, DS).transpose(1, 0, 2)

    bias_host = ((b_ih + b_hh)[idx] * (SX * SW)).astype(np.float32)
    bias_host = np.ascontiguousarray(bias_host.reshape(1, G))

    xh = np.concatenate([input, h_prev], axis=1) * SX       # [B, ktot] scaled
    x16 = xh.astype(np.float16)
    c16 = c_prev.astype(np.float16)

    in_maps = []
    for c in range(n_cores):
        rows = slice(c * b_loc, (c + 1) * b_loc)
        xc16 = x16[rows].T                                  # [ktot, b_loc]
        # [p, b, k, m] = x[k*128 + p, b*128 + m]
        xh16_h = xc16.reshape(KT16, 128, BT, 128).transpose(1, 2, 0, 3)
        in_maps.append({
            "xh16": np.ascontiguousarray(xh16_h),
            "w8": w8_host,
            "w16": w16_host,
            "bias": bias_host,
            "c_prev": np.ascontiguousarray(
                c16[rows].reshape(BT, 128, NQ, DS).transpose(1, 0, 2, 3)),
        })
    return in_maps


def unshard_output(results, b_loc=B_TOTAL // N_CORES, h_dim=H_DIM):
    """ch_out [BT, NQ, 128, 2, DS] tile-major -> full (h_t, c_t) fp32."""
    NQ = 4
    DS = h_dim // NQ
    BT = b_loc // 128
    chs = []
    for r in results:
        t = r["ch_out"].reshape(BT, NQ, 128, 2, DS)
        # -> [BT, 128, 2, NQ, DS] -> [b_loc, 2, h]
        chs.append(t.transpose(0, 2, 3, 1, 4).reshape(b_loc, 2, h_dim))
    ch = np.concatenate(chs, axis=0)
    c_t = ch[:, 0, :].astype(np.float32)
    h_t = ch[:, 1, :].astype(np.float32)
    return h_t, c_t


def run_lstm(inputs, trace=False, **spmd_kwargs):
    """Builds + runs the kernel on all 8 cores. Returns (h_t, c_t), results."""
    in_maps = prep_inputs(**inputs)
    nc = build_lstm_nc()
    res = run_bass_kernel_spmd(nc, in_maps, core_ids=list(range(N_CORES)),
                               trace=trace, **spmd_kwargs)
    h_t, c_t = unshard_output(res.results)
    return (h_t, c_t), res


def kernel(input, h_prev, c_prev, W_ih, b_ih, W_hh, b_hh):
    (h_t, c_t), _ = run_lstm(dict(
        input=input, h_prev=h_prev, c_prev=c_prev,
        W_ih=W_ih, b_ih=b_ih, W_hh=W_hh, b_hh=b_hh))
    return (h_t, c_t)


# revision 14
# speedup vs baseline: 1.2212x; 1.0609x over previous
"""DPLSTMCell Trainium2 kernel — per-gate mixed precision (fp8 + fp16).

Data-parallel LSTM cell over 8 NeuronCores: batch dim of input/h_prev/c_prev
is sharded, the (small) weights are replicated.

Precision scheme (error budget rel<2e-2; measured rel_h~=1.62e-2):
  gates i,f,o use fp8e4m3 DoubleRow matmuls (contract 256 k-rows per
  instruction instead of 128), the tanh-gate g stays fp16.  All-fp8
  measures 2.57e-2 (fails), so this mix is the PE-time floor: the PE
  streams 1 psum column per 2.4GHz cycle, giving
    rows = 8 b-tiles * (8*768 fp8 + 16*256 fp16) * 4 quarters
         = 327680 rows ~= 136.5us/core.
  Operands are pre-scaled by powers of two (x*8, W*512, exact in fp16) so
  fp8 stays clear of subnormals while psum+bias (max ~2e4) fits fp16:
  the gates tile is fp16, halving DVE/ACT width in the epilogue.  ACT
  descales by 2^-12 via its scale immediate.

Schedule (v5) — driven by NTFF traces (baseline 177.4us):
  - Quarter 0 was DMA-bound (~10MB of input in its ~35us window at
    ~330GB/s).  xh8 is not shipped at all: only xh16 loads; the DVE
    casts fp16->fp8e4 on-chip (~1.2us per b-tile, bit-exact RTN), saving
    2.1MB of head-critical DMA.  xh16 and the DoubleRow fp8 layout share
    the same linear SBUF layout so the cast is elementwise.
  - bias is loaded as one [1, 4H] f32 row (16KB) and partition-broadcast
    by the idle Pool engine, replacing a 2MB replicated load.
  - b0/b1 loads and casts are split in k-halves so the first real
    matmuls start as soon as ~700KB has landed.
  - Quarter 0 runs pairwise (GRP=2): with 4 PSUM buffers two pairs are
    in flight, so pair N+1's fp8 overlaps pair N's fp16/bias_add drain.
  - Engine instruction streams are in-order: a stalled op blocks
    everything behind it.  DMA dispatch order matches consumption order
    on both queues; the next quarter's W prefetch is emitted before this
    quarter's output DMAs on the sync queue; fp8 casts are emitted two
    pairs ahead on the DVE stream.
  - ch_out is tile-major in DRAM: each epilogue writes one contiguous
    128KB block (vs 256 strided 512B lines); host reassembles.
"""

import numpy as np
import ml_dtypes

import concourse.bacc as bacc
import concourse.mybir as mybir
import concourse.tile as tile
from concourse.bass_utils import run_bass_kernel_spmd

AF = mybir.ActivationFunctionType
DR = mybir.MatmulPerfMode.DoubleRow
F8 = mybir.dt.float8e4
F16 = mybir.dt.float16
F32 = mybir.dt.float32

N_CORES = 8
B_TOTAL = 8192
IN_DIM = 1024
H_DIM = 1024
P = 128

SX = 8.0         # x pre-scale (power of two)
SW = 512.0       # W pre-scale (power of two)
INV = 1.0 / (SX * SW)   # 2^-12, exact


def build_lstm_nc(b_loc=B_TOTAL // N_CORES, in_dim=IN_DIM, h_dim=H_DIM):
    ktot = in_dim + h_dim
    KT16 = ktot // P            # fp16 k-tiles (g gate)
    KT8 = ktot // (2 * P)       # fp8 DoubleRow k-super-tiles (i,f,o gates)
    G = 4 * h_dim               # total gate width
    NQ = 4                      # quarters, each [i|f|o|g] x DS
    QW = G // NQ                # quarter width (1024)
    DS = h_dim // NQ            # output-dim slice per quarter (256)
    W8C = 3 * DS                # fp8 cols per quarter (768: i,f,o)
    BT = b_loc // P             # batch tiles per core (8)

    nc = bacc.Bacc("TRN2", target_bir_lowering=False)
    # PE-ready host layouts; leading dim = SBUF partition (contraction k%128)
    xh16 = nc.dram_tensor("xh16", [P, BT, KT16, P], F16, kind="ExternalInput")
    w16 = nc.dram_tensor("w16", [NQ, P, KT16, DS], F16, kind="ExternalInput")
    w8 = nc.dram_tensor("w8", [NQ, P, KT8, 2, W8C], F8, kind="ExternalInput")
    bias = nc.dram_tensor("bias", [1, G], F32, kind="ExternalInput")
    c_prev = nc.dram_tensor("c_prev", [P, BT, NQ, DS], F16,
                            kind="ExternalInput")
    # tile-major output: epilogue (q,b) writes ch_out[b,q] as one
    # contiguous 128KB block; host reassembles to [b_loc, 2, h].
    ch_out = nc.dram_tensor("ch_out", [BT, NQ, P, 2, DS], F16,
                            kind="ExternalOutput")

    with tile.TileContext(nc) as tc:
        with (
            tc.tile_pool(name="const", bufs=1) as const_pool,
            tc.tile_pool(name="xh", bufs=1) as xh_pool,
            tc.tile_pool(name="w8p", bufs=2) as w8_pool,
            tc.tile_pool(name="w16p", bufs=2) as w16_pool,
            tc.tile_pool(name="work", bufs=3) as work,
            tc.tile_pool(name="psum", bufs=4, space="PSUM") as psum_pool,
        ):
            xh16_sb = xh_pool.tile([P, BT, KT16, P], F16)
            # fp8 copy of xh16, same linear layout; [:, b, 2t:2t+2, :] is
            # the DoubleRow stationary slice.
            xh8_sb = xh_pool.tile([P, BT, KT16, P], F8)
            cp_sb = xh_pool.tile([P, BT, NQ, DS], F16)
            bias_row = const_pool.tile([1, G], F32)
            bias_sb = const_pool.tile([P, G], F32)
            w8_tiles = {}
            w16_tiles = {}

            def alloc_w_quarter(q):
                w8_tiles[q] = w8_pool.tile([P, KT8, 2, W8C], F8, name="w8q")
                w16_tiles[q] = w16_pool.tile([P, KT16, DS], F16, name="w16q")

            def load_w_quarter(q):
                nc.sync.dma_start(w8_tiles[q][:], w8[q, :, :, :, :])
                nc.sync.dma_start(w16_tiles[q][:], w16[q, :, :, :])

            # ---- DMA dispatch plan ----------------------------------------
            # ONE queue (sync / hw DGE) with dispatches in exact consumption
            # order: a single queue reaches full DMA bandwidth, and the two
            # queues otherwise fight for the same 16 engines (and the sw-DGE
            # adds an expensive gpsimd drain at teardown).  Later quarters'
            # W is dispatched from inside the loop, after the head loads.
            alloc_w_quarter(0)
            nc.sync.dma_start(bias_row[:], bias[:, :])
            nc.sync.dma_start(xh16_sb[:, 0, 0:8], xh16[:, 0, 0:8])
            nc.sync.dma_start(w8_tiles[0][:, 0:2], w8[0, :, 0:2, :, :])
            nc.sync.dma_start(xh16_sb[:, 1, 0:8], xh16[:, 1, 0:8])
            nc.sync.dma_start(w8_tiles[0][:, 2:5], w8[0, :, 2:5, :, :])
            nc.sync.dma_start(xh16_sb[:, 0, 8:KT16], xh16[:, 0, 8:KT16])
            nc.sync.dma_start(xh16_sb[:, 1, 8:KT16], xh16[:, 1, 8:KT16])
            nc.sync.dma_start(w8_tiles[0][:, 5:KT8], w8[0, :, 5:KT8, :, :])
            nc.sync.dma_start(xh16_sb[:, 2], xh16[:, 2])
            nc.sync.dma_start(w16_tiles[0][:, 0:8], w16[0, :, 0:8, :])
            nc.sync.dma_start(xh16_sb[:, 3], xh16[:, 3])
            nc.sync.dma_start(w16_tiles[0][:, 8:KT16], w16[0, :, 8:KT16, :])
            nc.sync.dma_start(xh16_sb[:, 4:6], xh16[:, 4:6])
            nc.sync.dma_start(cp_sb[:, 0:2], c_prev[:, 0:2])
            nc.sync.dma_start(xh16_sb[:, 6:BT], xh16[:, 6:BT])
            nc.sync.dma_start(cp_sb[:, 2:5], c_prev[:, 2:5])
            nc.sync.dma_start(cp_sb[:, 5:BT], c_prev[:, 5:BT])

            # quarter-0 bias broadcast on the otherwise-idle Pool engine
            # (q1-3 slices are broadcast later, out of the critical window).
            nc.gpsimd.partition_broadcast(bias_sb[:, 0:QW],
                                          bias_row[0:1, 0:QW])

            # fp16->fp8 casts on the DVE (~1.2us per b-tile, bit-exact RTN),
            # emitted ahead of need; b0/b1 by k-halves for the fastest start.
            def cast_b(b, lo=0, hi=KT16):
                nc.vector.tensor_copy(xh8_sb[:, b, lo:hi],
                                      xh16_sb[:, b, lo:hi])

            cast_b(0, 0, 8)
            cast_b(1, 0, 8)
            cast_b(0, 8, KT16)
            cast_b(1, 8, KT16)
            cast_b(2)
            cast_b(3)

            # PE warmup: dummy matmuls on zeroed SBUF while the first W/xh
            # tiles stream in, so the PE p-state ramps before real work.
            scratch = work.tile([P, 512], F16, name="scratch", bufs=1)
            nc.vector.memset(scratch[:], 0.0)
            zb = const_pool.tile([P, 1], F32)
            nc.vector.memset(zb[:], 0.0)
            ps_w = psum_pool.tile([P, QW], F32, name="ps")
            for i in range(8):
                nc.tensor.matmul(
                    ps_w[:, (i % 2) * 512:(i % 2) * 512 + 512],
                    scratch[:, 0:P], scratch[:],
                    start=True, stop=True)

            def mm_fp8(ps, q, t, b):
                # i|f chunk (cols 0:512, psum bank A) and o chunk (512:768)
                w8_q = w8_tiles[q]
                xsl = xh8_sb[:, b, 2 * t:2 * t + 2, :]
                nc.tensor.matmul(ps[:, 0:512], xsl, w8_q[:, t, :, 0:512],
                                 perf_mode=DR,
                                 start=(t == 0), stop=(t == KT8 - 1))
                nc.tensor.matmul(ps[:, 512:W8C], xsl, w8_q[:, t, :, 512:W8C],
                                 perf_mode=DR,
                                 start=(t == 0), stop=(t == KT8 - 1))

            def mm_fp16(ps, q, k, b):
                # g chunk (cols 768:1024, psum bank B)
                nc.tensor.matmul(ps[:, W8C:QW],
                                 xh16_sb[:, b, k, :],
                                 w16_tiles[q][:, k, :],
                                 start=(k == 0), stop=(k == KT16 - 1))

            def epilogue(ps, q, b):
                # quarter layout: [ i | f | o | g ], each DS wide; the fp16
                # gates tile (values fit: max ~2e4 < 65504) halves DVE/ACT
                # width; ACT descales by 2^-12 via its scale immediate.
                # gates = psum + bias on the DVE; the ONLY psum reader, so
                # the PSUM slot frees right after it.
                gates = work_tile([P, QW], F16, "gates")
                nc.vector.tensor_add(
                    gates[:], ps[:], bias_sb[:, q * QW:(q + 1) * QW])
                act = work_tile([P, QW], F16, "act")
                nc.scalar.activation(act[:, 3 * DS:4 * DS],
                                     gates[:, 3 * DS:4 * DS], AF.Tanh,
                                     bias=zb[:], scale=INV)
                nc.scalar.activation(act[:, 0:3 * DS], gates[:, 0:3 * DS],
                                     AF.Sigmoid, bias=zb[:], scale=INV)

                ig = work_tile([P, DS], F16, "ig")
                nc.vector.tensor_mul(ig[:], act[:, 0:DS],
                                     act[:, 3 * DS:4 * DS])
                chnew = work_tile([P, 2, DS], F16, "chnew")
                cnew = chnew[:, 0, :]
                nc.vector.tensor_mul(cnew, act[:, DS:2 * DS],
                                     cp_sb[:, b, q, :])
                nc.vector.tensor_add(cnew, cnew, ig[:])
                tct = work_tile([P, DS], F16, "tct")
                nc.scalar.activation(tct[:], cnew, AF.Tanh, bias=zb[:])
                nc.vector.tensor_mul(chnew[:, 1, :], act[:, 2 * DS:3 * DS],
                                     tct[:])
                nc.sync.dma_start(ch_out[b, q], chnew[:, :, :])

            def work_tile(shape, dt, name):
                return work.tile(shape, dt, name=name, bufs=8)

            # ---- quarter 0: pairwise (2 psum tiles/pair, 4 bufs = 2 pairs
            # in flight): pair N+1's fp8 overlaps pair N's fp16 drain ----
            for g0 in range(0, BT, 2):
                pss = [psum_pool.tile([P, QW], F32, name="ps")
                       for _ in range(2)]
                for t in range(KT8):
                    for bi, ps in enumerate(pss):
                        mm_fp8(ps, 0, t, g0 + bi)
                for bi, ps in enumerate(pss):
                    for k in range(KT16):
                        mm_fp16(ps, 0, k, g0 + bi)
                # stay two pairs ahead on the fp8 casts (DVE in-order:
                # these run before the pair's epilogue adds)
                for b in range(g0 + 4, min(g0 + 6, BT)):
                    cast_b(b)
                if g0 == 2:
                    # quarter-1 W behind the head loads on the single queue
                    alloc_w_quarter(1)
                    load_w_quarter(1)
                    nc.gpsimd.partition_broadcast(bias_sb[:, QW:2 * QW],
                                                  bias_row[0:1, QW:2 * QW])
                for bi, ps in enumerate(pss):
                    epilogue(ps, 0, g0 + bi)

            # ---- quarters 1..3: prefetched, dense per-b chains; next
            # quarter's W dispatch is emitted BEFORE this quarter's output
            # DMAs so the sync engine's in-order stream never blocks the
            # prefetch behind epilogue semaphores ----
            for q in range(1, NQ):
                if q + 1 < NQ:
                    alloc_w_quarter(q + 1)
                    load_w_quarter(q + 1)
                    nc.gpsimd.partition_broadcast(
                        bias_sb[:, (q + 1) * QW:(q + 2) * QW],
                        bias_row[0:1, (q + 1) * QW:(q + 2) * QW])
                for b in range(BT):
                    ps = psum_pool.tile([P, QW], F32, name="ps")
                    for t in range(KT8):
                        mm_fp8(ps, q, t, b)
                    for k in range(KT16):
                        mm_fp16(ps, q, k, b)
                    epilogue(ps, q, b)

    nc.compile()
    return nc


def prep_inputs(input, h_prev, c_prev, W_ih, b_ih, W_hh, b_hh,
                n_cores=N_CORES):
    """Host-side shard + quantize + layout prep. Per-core input maps."""
    input = np.asarray(input, np.float32)
    h_prev = np.asarray(h_prev, np.float32)
    c_prev = np.asarray(c_prev, np.float32)
    W_ih = np.asarray(W_ih, np.float32)
    W_hh = np.asarray(W_hh, np.float32)
    b_ih = np.asarray(b_ih, np.float32)
    b_hh = np.asarray(b_hh, np.float32)

    b_total, in_dim = input.shape
    h_dim = h_prev.shape[1]
    ktot = in_dim + h_dim
    b_loc = b_total // n_cores
    G = 4 * h_dim
    NQ = 4
    DS = h_dim // NQ
    W8C = 3 * DS
    BT = b_loc // 128
    KT16 = ktot // 128
    KT8 = ktot // 256

    def q8(x):
        return np.clip(x, -240, 240).astype(ml_dtypes.float8_e4m3)

    # column reorder: per quarter q the layout is [i | f | o | g] for output
    # dims [q*DS, (q+1)*DS)
    arr = np.arange(G).reshape(4, NQ, DS)       # [gate, q, r]
    idx = arr[[0, 1, 3, 2]].transpose(1, 0, 2).reshape(-1)

    W_cat = np.concatenate([W_ih, W_hh], axis=1)[idx, :]    # [G, ktot] scaled
    Ws = W_cat * SW
    # fp8 blocks (i,f,o = first 768 cols of each quarter) in DoubleRow layout
    w8_host = np.empty((NQ, 128, KT8, 2, W8C), ml_dtypes.float8_e4m3)
    w16_host = np.empty((NQ, 128, KT16, DS), np.float16)
    for q in range(NQ):
        blk8 = q8(Ws[q * 1024:q * 1024 + W8C, :]).T         # [ktot, 768]
        w8_host[q] = blk8.reshape(KT8, 2, 128, W8C).transpose(2, 0, 1, 3)
        blk16 = Ws[q * 1024 + W8C:(q + 1) * 1024, :].T.astype(np.float16)
        w16_host[q] = blk16.reshape(KT16, 128, DS).transpose(1, 0, 2)

    bias_host = ((b_ih + b_hh)[idx] * (SX * SW)).astype(np.float32)
    bias_host = np.ascontiguousarray(bias_host.reshape(1, G))

    xh = np.concatenate([input, h_prev], axis=1) * SX       # [B, ktot] scaled
    x16 = xh.astype(np.float16)
    c16 = c_prev.astype(np.float16)

    in_maps = []
    for c in range(n_cores):
        rows = slice(c * b_loc, (c + 1) * b_loc)
        xc16 = x16[rows].T                                  # [ktot, b_loc]
        # [p, b, k, m] = x[k*128 + p, b*128 + m]
        xh16_h = xc16.reshape(KT16, 128, BT, 128).transpose(1, 2, 0, 3)
        in_maps.append({
            "xh16": np.ascontiguousarray(xh16_h),
            "w8": w8_host,
            "w16": w16_host,
            "bias": bias_host,
            "c_prev": np.ascontiguousarray(
                c16[rows].reshape(BT, 128, NQ, DS).transpose(1, 0, 2, 3)),
        })
    return in_maps


def unshard_output(results, b_loc=B_TOTAL // N_CORES, h_dim=H_DIM):
    """ch_out [BT, NQ, 128, 2, DS] tile-major -> full (h_t, c_t) fp32."""
    NQ = 4
    DS = h_dim // NQ
    BT = b_loc // 128
    chs = []
    for r in results:
        t = r["ch_out"].reshape(BT, NQ, 128, 2, DS)
        # -> [BT, 128, 2, NQ, DS] -> [b_loc, 2, h]
        chs.append(t.transpose(0, 2, 3, 1, 4).reshape(b_loc, 2, h_dim))
    ch = np.concatenate(chs, axis=0)
    c_t = ch[:, 0, :].astype(np.float32)
    h_t = ch[:, 1, :].astype(np.float32)
    return h_t, c_t


def run_lstm(inputs, trace=False, **spmd_kwargs):
    """Builds + runs the kernel on all 8 cores. Returns (h_t, c_t), results."""
    in_maps = prep_inputs(**inputs)
    nc = build_lstm_nc()
    res = run_bass_kernel_spmd(nc, in_maps, core_ids=list(range(N_CORES)),
                               trace=trace, **spmd_kwargs)
    h_t, c_t = unshard_output(res.results)
    return (h_t, c_t), res


def kernel(input, h_prev, c_prev, W_ih, b_ih, W_hh, b_hh):
    (h_t, c_t), _ = run_lstm(dict(
        input=input, h_prev=h_prev, c_prev=c_prev,
        W_ih=W_ih, b_ih=b_ih, W_hh=W_hh, b_hh=b_hh))
    return (h_t, c_t)


# revision 19
# speedup vs baseline: 1.2373x; 1.0132x over previous
"""DPLSTMCell Trainium2 kernel — per-gate mixed precision (fp8 + fp16).

Data-parallel LSTM cell over 8 NeuronCores: batch dim of input/h_prev/c_prev
is sharded, the (small) weights are replicated.

Precision scheme (error budget rel<2e-2; measured rel_h~=1.62e-2):
  gates i,f,o use fp8e4m3 DoubleRow matmuls (contract 256 k-rows per
  instruction instead of 128), the tanh-gate g stays fp16.  All-fp8
  measures 2.57e-2 (fails), so this mix is the PE-time floor: the PE
  streams 1 psum column per 2.4GHz cycle, giving
    rows = 8 b-tiles * (8*768 fp8 + 16*256 fp16) * 4 quarters
         = 327680 rows ~= 136.5us/core.
  Operands are pre-scaled by powers of two (x*8, W*512, exact in fp16) so
  fp8 stays clear of subnormals while psum+bias (max ~2e4) fits fp16:
  the gates tile is fp16, halving DVE/ACT width in the epilogue.  ACT
  descales by 2^-12 via its scale immediate.

Schedule (v5) — driven by NTFF traces (baseline 177.4us):
  - Quarter 0 was DMA-bound (~10MB of input in its ~35us window at
    ~330GB/s).  xh8 is not shipped at all: only xh16 loads; the DVE
    casts fp16->fp8e4 on-chip (~1.2us per b-tile, bit-exact RTN), saving
    2.1MB of head-critical DMA.  xh16 and the DoubleRow fp8 layout share
    the same linear SBUF layout so the cast is elementwise.
  - bias is loaded as one [1, 4H] f32 row (16KB) and partition-broadcast
    by the idle Pool engine, replacing a 2MB replicated load.
  - b0/b1 loads and casts are split in k-halves so the first real
    matmuls start as soon as ~700KB has landed.
  - Quarter 0 runs pairwise (GRP=2): with 4 PSUM buffers two pairs are
    in flight, so pair N+1's fp8 overlaps pair N's fp16/bias_add drain.
  - Engine instruction streams are in-order: a stalled op blocks
    everything behind it.  DMA dispatch order matches consumption order
    on both queues; the next quarter's W prefetch is emitted before this
    quarter's output DMAs on the sync queue; fp8 casts are emitted two
    pairs ahead on the DVE stream.
  - ch_out is tile-major in DRAM: each epilogue writes one contiguous
    128KB block (vs 256 strided 512B lines); host reassembles.
"""

import numpy as np
import ml_dtypes

import concourse.bacc as bacc
import concourse.mybir as mybir
import concourse.tile as tile
from concourse.bass_utils import run_bass_kernel_spmd

AF = mybir.ActivationFunctionType
DR = mybir.MatmulPerfMode.DoubleRow
F8 = mybir.dt.float8e4
F16 = mybir.dt.float16
F32 = mybir.dt.float32

N_CORES = 8
B_TOTAL = 8192
IN_DIM = 1024
H_DIM = 1024
P = 128

SX = 8.0         # x pre-scale (power of two)
SW = 512.0       # W pre-scale (power of two)
INV = 1.0 / (SX * SW)   # 2^-12, exact


def build_lstm_nc(b_loc=B_TOTAL // N_CORES, in_dim=IN_DIM, h_dim=H_DIM):
    ktot = in_dim + h_dim
    KT16 = ktot // P            # fp16 k-tiles (g gate)
    KT8 = ktot // (2 * P)       # fp8 DoubleRow k-super-tiles (i,f,o gates)
    G = 4 * h_dim               # total gate width
    NQ = 4                      # quarters, each [i|f|o|g] x DS
    QW = G // NQ                # quarter width (1024)
    DS = h_dim // NQ            # output-dim slice per quarter (256)
    W8C = 3 * DS                # fp8 cols per quarter (768: i,f,o)
    BT = b_loc // P             # batch tiles per core (8)

    nc = bacc.Bacc("TRN2", target_bir_lowering=False)
    # PE-ready host layouts; leading dim = SBUF partition (contraction k%128)
    xh16 = nc.dram_tensor("xh16", [P, BT, KT16, P], F16, kind="ExternalInput")
    w16 = nc.dram_tensor("w16", [NQ, P, KT16, DS], F16, kind="ExternalInput")
    w8 = nc.dram_tensor("w8", [NQ, P, KT8, 2, W8C], F8, kind="ExternalInput")
    bias = nc.dram_tensor("bias", [1, G], F32, kind="ExternalInput")
    c_prev = nc.dram_tensor("c_prev", [P, BT, NQ, DS], F16,
                            kind="ExternalInput")
    # tile-major output: epilogue (q,b) writes ch_out[b,q] as one
    # contiguous 128KB block; host reassembles to [b_loc, 2, h].
    ch_out = nc.dram_tensor("ch_out", [BT, NQ, P, 2, DS], F16,
                            kind="ExternalOutput")

    with tile.TileContext(nc) as tc:
        with (
            tc.tile_pool(name="const", bufs=1) as const_pool,
            tc.tile_pool(name="xh", bufs=1) as xh_pool,
            tc.tile_pool(name="w8p", bufs=2) as w8_pool,
            tc.tile_pool(name="w16p", bufs=2) as w16_pool,
            tc.tile_pool(name="work", bufs=3) as work,
            tc.tile_pool(name="psum", bufs=4, space="PSUM") as psum_pool,
        ):
            xh16_sb = xh_pool.tile([P, BT, KT16, P], F16)
            # fp8 copy of xh16, same linear layout; [:, b, 2t:2t+2, :] is
            # the DoubleRow stationary slice.
            xh8_sb = xh_pool.tile([P, BT, KT16, P], F8)
            cp_sb = xh_pool.tile([P, BT, NQ, DS], F16)
            bias_row = const_pool.tile([1, G], F32)
            bias_sb = const_pool.tile([P, G], F32)
            w8_tiles = {}
            w16_tiles = {}

            def alloc_w_quarter(q):
                w8_tiles[q] = w8_pool.tile([P, KT8, 2, W8C], F8, name="w8q")
                w16_tiles[q] = w16_pool.tile([P, KT16, DS], F16, name="w16q")

            def load_w_quarter(q):
                nc.sync.dma_start(w8_tiles[q][:], w8[q, :, :, :, :])
                nc.sync.dma_start(w16_tiles[q][:], w16[q, :, :, :])

            # ---- DMA dispatch plan ----------------------------------------
            # ONE queue (sync / hw DGE) with dispatches in exact consumption
            # order: a single queue reaches full DMA bandwidth, and the two
            # queues otherwise fight for the same 16 engines (and the sw-DGE
            # adds an expensive gpsimd drain at teardown).  Later quarters'
            # W is dispatched from inside the loop, after the head loads.
            alloc_w_quarter(0)
            nc.sync.dma_start(bias_row[:], bias[:, :])
            nc.sync.dma_start(xh16_sb[:, 0, 0:8], xh16[:, 0, 0:8])
            nc.sync.dma_start(w8_tiles[0][:, 0:2], w8[0, :, 0:2, :, :])
            nc.sync.dma_start(xh16_sb[:, 1, 0:8], xh16[:, 1, 0:8])
            nc.sync.dma_start(w8_tiles[0][:, 2:5], w8[0, :, 2:5, :, :])
            nc.sync.dma_start(xh16_sb[:, 2, 0:8], xh16[:, 2, 0:8])
            nc.sync.dma_start(w8_tiles[0][:, 5:KT8], w8[0, :, 5:KT8, :, :])
            nc.sync.dma_start(xh16_sb[:, 3, 0:8], xh16[:, 3, 0:8])
            nc.sync.dma_start(xh16_sb[:, 2, 8:KT16], xh16[:, 2, 8:KT16])
            nc.sync.dma_start(xh16_sb[:, 3, 8:KT16], xh16[:, 3, 8:KT16])
            nc.sync.dma_start(w16_tiles[0][:, 0:8], w16[0, :, 0:8, :])
            nc.sync.dma_start(xh16_sb[:, 0, 8:KT16], xh16[:, 0, 8:KT16])
            nc.sync.dma_start(xh16_sb[:, 1, 8:KT16], xh16[:, 1, 8:KT16])
            nc.sync.dma_start(w16_tiles[0][:, 8:KT16], w16[0, :, 8:KT16, :])
            nc.sync.dma_start(xh16_sb[:, 4:6], xh16[:, 4:6])
            nc.sync.dma_start(cp_sb[:, 0:2], c_prev[:, 0:2])
            nc.sync.dma_start(xh16_sb[:, 6:BT], xh16[:, 6:BT])
            nc.sync.dma_start(cp_sb[:, 2:5], c_prev[:, 2:5])
            nc.sync.dma_start(cp_sb[:, 5:BT], c_prev[:, 5:BT])

            # quarter-0 bias broadcast on the otherwise-idle Pool engine
            # (q1-3 slices are broadcast later, out of the critical window).
            nc.gpsimd.partition_broadcast(bias_sb[:, 0:QW],
                                          bias_row[0:1, 0:QW])

            # fp16->fp8 casts on the DVE (~1.2us per b-tile, bit-exact RTN),
            # emitted ahead of need; b0/b1 by k-halves for the fastest start.
            def cast_b(b, lo=0, hi=KT16):
                nc.vector.tensor_copy(xh8_sb[:, b, lo:hi],
                                      xh16_sb[:, b, lo:hi])

            cast_b(0, 0, 8)
            cast_b(1, 0, 8)
            cast_b(2, 0, 8)
            cast_b(3, 0, 8)
            cast_b(2, 8, KT16)
            cast_b(3, 8, KT16)
            cast_b(0, 8, KT16)
            cast_b(1, 8, KT16)

            # PE warmup: dummy matmuls on zeroed SBUF while the first W/xh
            # tiles stream in, so the PE p-state ramps before real work.
            scratch = work.tile([P, 512], F16, name="scratch", bufs=1)
            nc.vector.memset(scratch[:], 0.0)
            zb = const_pool.tile([P, 1], F32)
            nc.vector.memset(zb[:], 0.0)
            ps_w = psum_pool.tile([P, QW], F32, name="ps")
            for i in range(8):
                nc.tensor.matmul(
                    ps_w[:, (i % 2) * 512:(i % 2) * 512 + 512],
                    scratch[:, 0:P], scratch[:],
                    start=True, stop=True)

            def mm_fp8(ps, q, t, b):
                # i|f chunk (cols 0:512, psum bank A) and o chunk (512:768)
                w8_q = w8_tiles[q]
                xsl = xh8_sb[:, b, 2 * t:2 * t + 2, :]
                nc.tensor.matmul(ps[:, 0:512], xsl, w8_q[:, t, :, 0:512],
                                 perf_mode=DR,
                                 start=(t == 0), stop=(t == KT8 - 1))
                nc.tensor.matmul(ps[:, 512:W8C], xsl, w8_q[:, t, :, 512:W8C],
                                 perf_mode=DR,
                                 start=(t == 0), stop=(t == KT8 - 1))

            def mm_fp16(ps, q, k, b):
                # g chunk (cols 768:1024, psum bank B)
                nc.tensor.matmul(ps[:, W8C:QW],
                                 xh16_sb[:, b, k, :],
                                 w16_tiles[q][:, k, :],
                                 start=(k == 0), stop=(k == KT16 - 1))

            def epilogue(ps, q, b, fast_tail=False):
                # quarter layout: [ i | f | o | g ], each DS wide; the fp16
                # gates tile (values fit: max ~2e4 < 65504) halves DVE/ACT
                # width; ACT descales by 2^-12 via its scale immediate.
                # gates = psum + bias on the DVE; the ONLY psum reader, so
                # the PSUM slot frees right after it.  fast_tail (final
                # tiles) splits the adds/DMA so the chain latency after the
                # last matmul shrinks.
                gates = work_tile([P, QW], F16, "gates")
                act = work_tile([P, QW], F16, "act")
                if fast_tail:
                    nc.vector.tensor_add(
                        gates[:, 3 * DS:QW], ps[:, 3 * DS:QW],
                        bias_sb[:, q * QW + 3 * DS:(q + 1) * QW])
                    nc.scalar.activation(act[:, 3 * DS:4 * DS],
                                         gates[:, 3 * DS:4 * DS], AF.Tanh,
                                         bias=zb[:], scale=INV)
                    nc.vector.tensor_add(
                        gates[:, 0:3 * DS], ps[:, 0:3 * DS],
                        bias_sb[:, q * QW:q * QW + 3 * DS])
                    nc.scalar.activation(act[:, 0:3 * DS], gates[:, 0:3 * DS],
                                         AF.Sigmoid, bias=zb[:], scale=INV)
                else:
                    nc.vector.tensor_add(
                        gates[:], ps[:], bias_sb[:, q * QW:(q + 1) * QW])
                    nc.scalar.activation(act[:, 3 * DS:4 * DS],
                                         gates[:, 3 * DS:4 * DS], AF.Tanh,
                                         bias=zb[:], scale=INV)
                    nc.scalar.activation(act[:, 0:3 * DS], gates[:, 0:3 * DS],
                                         AF.Sigmoid, bias=zb[:], scale=INV)

                ig = work_tile([P, DS], F16, "ig")
                nc.vector.tensor_mul(ig[:], act[:, 0:DS],
                                     act[:, 3 * DS:4 * DS])
                chnew = work_tile([P, 2, DS], F16, "chnew")
                cnew = chnew[:, 0, :]
                nc.vector.tensor_mul(cnew, act[:, DS:2 * DS],
                                     cp_sb[:, b, q, :])
                nc.vector.tensor_add(cnew, cnew, ig[:])
                tct = work_tile([P, DS], F16, "tct")
                nc.scalar.activation(tct[:], cnew, AF.Tanh, bias=zb[:])
                nc.vector.tensor_mul(chnew[:, 1, :], act[:, 2 * DS:3 * DS],
                                     tct[:])
                if fast_tail:
                    nc.sync.dma_start(ch_out[b, q, :, 0:1, :],
                                      chnew[:, 0:1, :])
                    nc.sync.dma_start(ch_out[b, q, :, 1:2, :],
                                      chnew[:, 1:2, :])
                else:
                    nc.sync.dma_start(ch_out[b, q], chnew[:, :, :])

            def work_tile(shape, dt, name):
                return work.tile(shape, dt, name=name, bufs=8)

            # ---- quarter 0: pairwise (2 psum tiles/pair, 4 bufs = 2 pairs
            # in flight).  Pairs 0 and 1 are staggered: both fp8 phases run
            # before pair 0's fp16, so the PE streams fp8 (fed by the small
            # early loads) while w16-q0 and the xh16 k-hi halves arrive ----
            def pair_fp8(pss, g0):
                for t in range(KT8):
                    for bi, ps in enumerate(pss):
                        mm_fp8(ps, 0, t, g0 + bi)

            def pair_fp16_epi(pss, g0):
                for bi, ps in enumerate(pss):
                    for k in range(KT16):
                        mm_fp16(ps, 0, k, g0 + bi)
                for bi, ps in enumerate(pss):
                    epilogue(ps, 0, g0 + bi)

            ps01 = [psum_pool.tile([P, QW], F32, name="ps") for _ in range(2)]
            pair_fp8(ps01, 0)
            ps23 = [psum_pool.tile([P, QW], F32, name="ps") for _ in range(2)]
            pair_fp8(ps23, 2)
            cast_b(4)
            cast_b(5)
            pair_fp16_epi(ps01, 0)
            cast_b(6)
            cast_b(7)
            # quarter-1 W behind the head loads on the single queue
            alloc_w_quarter(1)
            load_w_quarter(1)
            nc.gpsimd.partition_broadcast(bias_sb[:, QW:2 * QW],
                                          bias_row[0:1, QW:2 * QW])
            pair_fp16_epi(ps23, 2)
            for g0 in (4, 6):
                pss = [psum_pool.tile([P, QW], F32, name="ps")
                       for _ in range(2)]
                pair_fp8(pss, g0)
                pair_fp16_epi(pss, g0)

            # ---- quarters 1..3: prefetched, dense per-b chains; next
            # quarter's W dispatch is emitted BEFORE this quarter's output
            # DMAs so the sync engine's in-order stream never blocks the
            # prefetch behind epilogue semaphores ----
            for q in range(1, NQ):
                if q + 1 < NQ:
                    alloc_w_quarter(q + 1)
                    load_w_quarter(q + 1)
                    nc.gpsimd.partition_broadcast(
                        bias_sb[:, (q + 1) * QW:(q + 2) * QW],
                        bias_row[0:1, (q + 1) * QW:(q + 2) * QW])
                for b in range(BT):
                    ps = psum_pool.tile([P, QW], F32, name="ps")
                    for t in range(KT8):
                        mm_fp8(ps, q, t, b)
                    for k in range(KT16):
                        mm_fp16(ps, q, k, b)
                    epilogue(ps, q, b,
                             fast_tail=(q == NQ - 1 and b >= BT - 2))

    nc.compile()
    return nc


def prep_inputs(input, h_prev, c_prev, W_ih, b_ih, W_hh, b_hh,
                n_cores=N_CORES):
    """Host-side shard + quantize + layout prep. Per-core input maps."""
    input = np.asarray(input, np.float32)
    h_prev = np.asarray(h_prev, np.float32)
    c_prev = np.asarray(c_prev, np.float32)
    W_ih = np.asarray(W_ih, np.float32)
    W_hh = np.asarray(W_hh, np.float32)
    b_ih = np.asarray(b_ih, np.float32)
    b_hh = np.asarray(b_hh, np.float32)

    b_total, in_dim = input.shape
    h_dim = h_prev.shape[1]
    ktot = in_dim + h_dim
    b_loc = b_total // n_cores
    G = 4 * h_dim
    NQ = 4
    DS = h_dim // NQ
    W8C = 3 * DS
    BT = b_loc // 128
    KT16 = ktot // 128
    KT8 = ktot // 256

    def q8(x):
        return np.clip(x, -240, 240).astype(ml_dtypes.float8_e4m3)

    # column reorder: per quarter q the layout is [i | f | o | g] for output
    # dims [q*DS, (q+1)*DS)
    arr = np.arange(G).reshape(4, NQ, DS)       # [gate, q, r]
    idx = arr[[0, 1, 3, 2]].transpose(1, 0, 2).reshape(-1)

    W_cat = np.concatenate([W_ih, W_hh], axis=1)[idx, :]    # [G, ktot] scaled
    Ws = W_cat * SW
    # fp8 blocks (i,f,o = first 768 cols of each quarter) in DoubleRow layout
    w8_host = np.empty((NQ, 128, KT8, 2, W8C), ml_dtypes.float8_e4m3)
    w16_host = np.empty((NQ, 128, KT16, DS), np.float16)
    for q in range(NQ):
        blk8 = q8(Ws[q * 1024:q * 1024 + W8C, :]).T         # [ktot, 768]
        w8_host[q] = blk8.reshape(KT8, 2, 128, W8C).transpose(2, 0, 1, 3)
        blk16 = Ws[q * 1024 + W8C:(q + 1) * 1024, :].T.astype(np.float16)
        w16_host[q] = blk16.reshape(KT16, 128, DS).transpose(1, 0, 2)

    bias_host = ((b_ih + b_hh)[idx] * (SX * SW)).astype(np.float32)
    bias_host = np.ascontiguousarray(bias_host.reshape(1, G))

    xh = np.concatenate([input, h_prev], axis=1) * SX       # [B, ktot] scaled
    x16 = xh.astype(np.float16)
    c16 = c_prev.astype(np.float16)

    in_maps = []
    for c in range(n_cores):
        rows = slice(c * b_loc, (c + 1) * b_loc)
        xc16 = x16[rows].T                                  # [ktot, b_loc]
        # [p, b, k, m] = x[k*128 + p, b*128 + m]
        xh16_h = xc16.reshape(KT16, 128, BT, 128).transpose(1, 2, 0, 3)
        in_maps.append({
            "xh16": np.ascontiguousarray(xh16_h),
            "w8": w8_host,
            "w16": w16_host,
            "bias": bias_host,
            "c_prev": np.ascontiguousarray(
                c16[rows].reshape(BT, 128, NQ, DS).transpose(1, 0, 2, 3)),
        })
    return in_maps


def unshard_output(results, b_loc=B_TOTAL // N_CORES, h_dim=H_DIM):
    """ch_out [BT, NQ, 128, 2, DS] tile-major -> full (h_t, c_t) fp32."""
    NQ = 4
    DS = h_dim // NQ
    BT = b_loc // 128
    chs = []
    for r in results:
        t = r["ch_out"].reshape(BT, NQ, 128, 2, DS)
        # -> [BT, 128, 2, NQ, DS] -> [b_loc, 2, h]
        chs.append(t.transpose(0, 2, 3, 1, 4).reshape(b_loc, 2, h_dim))
    ch = np.concatenate(chs, axis=0)
    c_t = ch[:, 0, :].astype(np.float32)
    h_t = ch[:, 1, :].astype(np.float32)
    return h_t, c_t


def run_lstm(inputs, trace=False, **spmd_kwargs):
    """Builds + runs the kernel on all 8 cores. Returns (h_t, c_t), results."""
    in_maps = prep_inputs(**inputs)
    nc = build_lstm_nc()
    res = run_bass_kernel_spmd(nc, in_maps, core_ids=list(range(N_CORES)),
                               trace=trace, **spmd_kwargs)
    h_t, c_t = unshard_output(res.results)
    return (h_t, c_t), res


def kernel(input, h_prev, c_prev, W_ih, b_ih, W_hh, b_hh):
    (h_t, c_t), _ = run_lstm(dict(
        input=input, h_prev=h_prev, c_prev=c_prev,
        W_ih=W_ih, b_ih=b_ih, W_hh=W_hh, b_hh=b_hh))
    return (h_t, c_t)
